# revision 1
# baseline (speedup 1.0000x reference)
"""Trainium2 Bass kernel for nn_DiT_18056042512615.

DiT block on voxel latents: adaLN-modulated snorm -> 4-head attention ->
residual -> adaLN-modulated snorm -> residual (ffn is dead in the source).

Sharding: pure data parallel over ZN (batch) - 64 samples / 8 cores =
8 samples per core; all weights replicated.

v2 design notes (vs the 380us baseline):
- All large matmuls run in bf16 (1 col/cycle on the PE; the f32r path
  measured ~3x slower per column on HW). Tolerance is 2e-2 so bf16
  noise (~1e-3 on the output) is fine.
- Attention: S^T per chunk is 4 row-tiled MMs (one per head, 32-row
  groups, concurrent on the PE). exp runs as ONE [128, 2048] ACTIVATE
  over all 4 heads of a chunk (amortizes the ~350-cycle ACT overhead),
  with 1/sqrt(dk) folded into the activation's free scale. P@V and the
  softmax denominator are 4-way col-tiled MM groups accumulating over
  chunks; the denominator lands partition-aligned with P@V rows so one
  reciprocal + one multiply normalizes all 4 heads at once.
- ACT table sets: exp and ln are pinned to the combined
  natural_log_exp_and_others set (the default chooser put them in
  different sets -> 33 table loads x 1.3us in the baseline). rstd =
  exp(-0.5*ln(v)) stays, with the exp batched over sample pairs.
- Elementwise norm chain runs bf16-in/bf16-out in SBUF (4x DVE mode);
  all Identity bias-applies moved from ACT (the bottleneck) to DVE.
- Emission is software-pipelined over sample pairs so the ACT queue
  (strict FIFO) never waits on work emitted later.
"""

import sys

import numpy as np

try:
    import concourse.bass as bass
except ImportError:  # container fallback path
    sys.path.insert(0, "/opt/trn_rl_repo")
    import concourse.bass as bass

import concourse.tile as tile
from concourse import bacc, bass_isa, mybir
from concourse.bass_utils import run_bass_kernel_spmd

F32 = mybir.dt.float32
F32R = mybir.dt.float32r
BF16 = mybir.dt.bfloat16

D = 128        # model dim
H = 4          # heads
DK = 32        # head dim
ZN = 64        # batch (full)
NCORES = 8
SPC = ZN // NCORES   # samples per core
N = 512        # tokens per sample (8*8*8)
NC = 128       # tokens per chunk
AF = mybir.ActivationFunctionType
ALU = mybir.AluOpType

Q_SCALE = 1.0 / (DK ** 0.5)

_MLP_PRES = (("an_gb", 2), ("an_a", 1), ("fn_gb", 2), ("fn_a", 1))

_WEIGHT_NAMES = [
    "qw", "kw", "vw", "qb", "kb", "vb", "ow",
]
for _pre in ("an_gb", "an_a", "fn_gb", "fn_a"):
    for _suf in ("w1", "b1", "w2", "b2", "w3", "b3"):
        _WEIGHT_NAMES.append(f"{_pre}_{_suf}")


def _patch_act_tables():
    """Pin Exp and Ln to the combined natural_log_exp_and_others table
    set so the whole kernel needs a single ACT_TABLE_LOAD. The default
    chooser picks the first set containing each function (exp_and_others
    for Exp, natural_log for Ln), which forces a ~1.3us table swap at
    every Ln<->Exp transition. Only affects compilation in this process.
    """
    import functools

    from concourse import bass_interp, hw_specs
    from concourse import bacc as bacc_mod

    orig = hw_specs.get_activation_tables.__wrapped__

    @functools.cache
    def patched(arch):
        out = {}
        for name, funcs in orig(arch).items():
            fs = set(funcs)
            if name != "natural_log_exp_and_others":
                fs.discard(AF.Exp)
                fs.discard(AF.Ln)
            out[name] = fs
        return out

    hw_specs.get_activation_tables = patched
    bacc_mod.get_activation_tables = patched
    bass_interp.get_activation_tables = patched


def build_program():
    """Build the per-core SPMD Bass program. Identical on all 8 cores."""
    _patch_act_tables()
    nc = bacc.Bacc("TRN2", target_bir_lowering=False, debug=False)

    lat = nc.dram_tensor("latent", [SPC, D, 8, 8, 8], F32, kind="ExternalInput").ap()
    nodes = nc.dram_tensor("nodes", [SPC, D], F32, kind="ExternalInput").ap()
    t_in = nc.dram_tensor("t", [SPC], F32, kind="ExternalInput").ap()
    w = {}
    w["qw"] = nc.dram_tensor("qw", [H, D, DK], F32, kind="ExternalInput").ap()
    w["kw"] = nc.dram_tensor("kw", [H, D, DK], F32, kind="ExternalInput").ap()
    w["vw"] = nc.dram_tensor("vw", [H, D, DK], F32, kind="ExternalInput").ap()
    w["qb"] = nc.dram_tensor("qb", [H, DK], F32, kind="ExternalInput").ap()
    w["kb"] = nc.dram_tensor("kb", [H, DK], F32, kind="ExternalInput").ap()
    w["vb"] = nc.dram_tensor("vb", [H, DK], F32, kind="ExternalInput").ap()
    w["ow"] = nc.dram_tensor("ow", [D, D], F32, kind="ExternalInput").ap()
    for pre, dout in (("an_gb", 2 * D), ("an_a", D), ("fn_gb", 2 * D), ("fn_a", D)):
        w[pre + "_w1"] = nc.dram_tensor(pre + "_w1", [D, D], F32, kind="ExternalInput").ap()
        w[pre + "_b1"] = nc.dram_tensor(pre + "_b1", [D], F32, kind="ExternalInput").ap()
        w[pre + "_w2"] = nc.dram_tensor(pre + "_w2", [D, D], F32, kind="ExternalInput").ap()
        w[pre + "_b2"] = nc.dram_tensor(pre + "_b2", [D], F32, kind="ExternalInput").ap()
        w[pre + "_w3"] = nc.dram_tensor(pre + "_w3", [D, dout], F32, kind="ExternalInput").ap()
        w[pre + "_b3"] = nc.dram_tensor(pre + "_b3", [dout], F32, kind="ExternalInput").ap()
    out = nc.dram_tensor("out", [SPC, D, 8, 8, 8], F32, kind="ExternalOutput").ap()

    lat2 = lat.rearrange("s d a b c -> s d (a b c)")     # [SPC, 128, 512]
    out2 = out.rearrange("s d a b c -> s d (a b c)")

    with tile.TileContext(nc) as tc:
        _body(nc, tc, lat2, nodes, t_in, w, out2)
    nc.compile()
    return nc


def _body(nc, tc, lat2, nodes, t_in, w, out2):
    import contextlib
    ctx = contextlib.ExitStack()
    with ctx:
        wp = ctx.enter_context(tc.tile_pool(name="weights", bufs=1))
        mlp_tmp = ctx.enter_context(tc.tile_pool(name="mlp_tmp", bufs=4))

        xt_p = ctx.enter_context(tc.tile_pool(name="xt", bufs=8))
        xc_p = ctx.enter_context(tc.tile_pool(name="xc", bufs=4))
        xsq_p = ctx.enter_context(tc.tile_pool(name="xsq", bufs=3))
        lnp_p = ctx.enter_context(tc.tile_pool(name="lnp", bufs=3))
        rstd_p = ctx.enter_context(tc.tile_pool(name="rstd", bufs=3))
        xh_p = ctx.enter_context(tc.tile_pool(name="xh", bufs=3))
        x2_p = ctx.enter_context(tc.tile_pool(name="x2", bufs=4))
        qt_p = ctx.enter_context(tc.tile_pool(name="qt", bufs=4))
        kt_p = ctx.enter_context(tc.tile_pool(name="kt", bufs=4))
        v_p = ctx.enter_context(tc.tile_pool(name="v", bufs=4))
        est_p = ctx.enter_context(tc.tile_pool(name="est", bufs=3))
        rd_p = ctx.enter_context(tc.tile_pool(name="rd", bufs=2))
        oall_p = ctx.enter_context(tc.tile_pool(name="oall", bufs=2))
        x1_p = ctx.enter_context(tc.tile_pool(name="x1", bufs=4))
        xf_p = ctx.enter_context(tc.tile_pool(name="xf", bufs=3))

        # PSUM: 8 banks. sp(2) + pv(1) + den(1) live throughout; mlp_ps(2)
        # only exists during startup and is released before st4(4) is
        # entered, so the ring reuses its banks (the overlap dep orders the
        # first S^T after the last MLP read, which is true anyway).
        sp = ctx.enter_context(tc.tile_pool(name="sp", bufs=2, space="PSUM"))
        pv_p = ctx.enter_context(tc.tile_pool(name="pv", bufs=1, space="PSUM"))
        den_p = ctx.enter_context(tc.tile_pool(name="den", bufs=1, space="PSUM"))
        mlp_ps_cm = tc.tile_pool(name="mlp_ps", bufs=2, space="PSUM")
        mlp_ps = mlp_ps_cm.__enter__()
        st4_holder = {}

        dma = nc.sync.dma_start
        wdma = nc.gpsimd.dma_start

        # ================= per-core constants =================
        onesmat_f = wp.tile([D, D], F32, tag="onesmat_f")
        nc.vector.memset(onesmat_f, 1.0)
        onesmat_r = wp.tile([D, D], F32R, tag="onesmat_r")
        nc.vector.tensor_copy(out=onesmat_r, in_=onesmat_f)
        ones_bf = wp.tile([D, D], BF16, tag="ones_bf")
        nc.vector.tensor_copy(out=ones_bf, in_=onesmat_f)
        warm_src = wp.tile([D, N], BF16, tag="warm_src")
        nc.vector.memset(warm_src, 0.5)

        def warm_pe(n_mm):
            """Dummy matmuls that keep the PE HAM activity window busy
            during DMA-bound phases so real matmuls run at 2.4 GHz."""
            for _ in range(n_mm):
                wm = sp.tile([D, N], F32, tag="sp", name="warm")
                nc.tensor.matmul(out=wm, lhsT=ones_bf, rhs=warm_src,
                                 skip_group_check=True)

        # qkv projection weights as [d, (h k)] in bf16 (loaded via
        # qkv_w dict; emission point controls the DMA queue order)
        qkv_w = {}

        def load_bf(name, src_ap):
            stage = mlp_tmp.tile([D, D], F32, tag=f"{name}_stage",
                                 name=f"{name}_stage")
            wdma(out=stage, in_=src_ap)
            t = wp.tile([D, D], BF16, tag=name, name=name)
            nc.vector.tensor_copy(out=t, in_=stage)
            return t

        def load_qkv_weights():
            qkv_w["qw"] = load_bf("qw", w["qw"].rearrange("h d k -> d h k"))
            qkv_w["kw"] = load_bf("kw", w["kw"].rearrange("h d k -> d h k"))
            qkv_w["vw"] = load_bf("vw", w["vw"].rearrange("h d k -> d h k"))
            # ow with rows permuted to match the (h,k)-ordered O we build
            # (reference concatenates heads interleaved: d' = k*H + h)
            qkv_w["ow"] = load_bf("ow", w["ow"].rearrange("(k h) j -> h k j", h=H))

            qb_sb = wp.tile([D, 1], F32, tag="qb", name="qb_sb")
            kb_sb = wp.tile([D, 1], F32, tag="kb", name="kb_sb")
            wdma(out=qb_sb, in_=w["qb"].rearrange("h k -> (h k)")[:, None])
            wdma(out=kb_sb, in_=w["kb"].rearrange("h k -> (h k)")[:, None])
            qkv_w["qb"], qkv_w["kb"] = qb_sb, kb_sb

            vb_row_f = wp.tile([1, D], F32, tag="vb_row_f", name="vb_row_f")
            wdma(out=vb_row_f, in_=w["vb"].rearrange("h k -> (h k)")[None, :])
            vb_row = wp.tile([1, D], BF16, tag="vb_row", name="vb_row")
            nc.vector.tensor_copy(out=vb_row, in_=vb_row_f)
            ones_row = wp.tile([1, D], BF16, tag="ones_row", name="ones_row")
            nc.vector.memset(ones_row, 1.0)
            vb_ps = sp.tile([D, D], F32, tag="sp", name="vb_ps")
            nc.tensor.matmul(out=vb_ps, lhsT=ones_row, rhs=vb_row,
                             skip_group_check=True)
            vb_b = wp.tile([D, D], F32, tag="vb_b", name="vb_b")
            nc.vector.tensor_copy(out=vb_b, in_=vb_ps)
            qkv_w["vb_b"] = vb_b

        # ================= cond MLPs =================
        # cond^T [d, s] = nodes^T + t. Computed as ONE small matmul
        # [nodes; ones]^T @ [I; t] instead of transpose/broadcast gather
        # DMAs (those cost ~4.4us of serial descriptor time on the sync
        # queue and starved the whole startup).
        nodes_f = wp.tile([SPC, D], F32, tag="nodes_f")
        dma(out=nodes_f, in_=nodes)
        t_f = wp.tile([1, SPC], F32, tag="t_f")
        dma(out=t_f, in_=t_in[None, :])
        cond_stage = wp.tile([SPC, D], BF16, tag="cond_stage")
        nc.vector.tensor_copy(out=cond_stage, in_=nodes_f)
        iota_t = wp.tile([SPC, SPC], mybir.dt.int32, tag="iota_t")
        nc.gpsimd.iota(iota_t, pattern=[[1, SPC]], base=0, channel_multiplier=-1)
        ident_bf = wp.tile([SPC, SPC], BF16, tag="ident_bf")
        nc.vector.tensor_scalar(out=ident_bf, in0=iota_t, scalar1=0,
                                scalar2=None, op0=ALU.is_equal)
        t_bf = wp.tile([1, SPC], BF16, tag="t_bf")
        nc.vector.tensor_copy(out=t_bf, in_=t_f)
        ones_r1 = wp.tile([1, D], BF16, tag="ones_r1")
        nc.vector.memset(ones_r1, 1.0)
        condT_ps = sp.tile([D, SPC], F32, tag="sp", name="condT_ps")
        nc.tensor.matmul(out=condT_ps, lhsT=cond_stage, rhs=ident_bf,
                         start=True, stop=False, skip_group_check=True)
        nc.tensor.matmul(out=condT_ps, lhsT=ones_r1, rhs=t_bf,
                         start=False, stop=True, skip_group_check=True)
        condT = wp.tile([D, SPC], F32, tag="condT")
        nc.vector.tensor_copy(out=condT, in_=condT_ps)

        def load_bias_col(name, lo=None):
            b = w[name]
            tl = wp.tile([D, 1], F32, tag=f"{name}_{lo}")
            src = b if lo is None else b[lo:lo + D]
            dma(out=tl, in_=src[:, None])
            return tl

        def load_xt(s):
            xt = xt_p.tile([D, N], F32R, tag="xt", name=f"xt_{s}")
            dma(out=xt, in_=lat2[s].bitcast(F32R))
            xts[s] = xt

        mlp_w = {}

        def load_mlp_weights():
            """MLP weight DMAs in layer-major order (all w1 first so layer 1
            can start ASAP), biases on the sync queue in parallel."""
            b1cat = wp.tile([D, 4], F32, tag="b1cat", name="b1cat")
            b2cat = wp.tile([D, 4], F32, tag="b2cat", name="b2cat")
            b3cat = wp.tile([D, 6], F32, tag="b3cat", name="b3cat")
            tiles = {}
            for pre, nout in _MLP_PRES:
                tiles[pre] = (
                    wp.tile([D, D], F32, tag=f"{pre}_w1", name=f"{pre}_w1"),
                    wp.tile([D, D], F32, tag=f"{pre}_w2", name=f"{pre}_w2"),
                    wp.tile([D, nout * D], F32, tag=f"{pre}_w3",
                            name=f"{pre}_w3"),
                )
            for li in range(3):
                for pre, nout in _MLP_PRES:
                    wdma(out=tiles[pre][li], in_=w[f"{pre}_{'w' + str(li + 1)}"])
            load_xt(0)
            load_xt(1)
            for pi, (pre, nout) in enumerate(_MLP_PRES):
                dma(out=b1cat[:, pi:pi + 1], in_=w[f"{pre}_b1"][:, None])
            load_xt(2)
            load_xt(3)
            for pi, (pre, nout) in enumerate(_MLP_PRES):
                dma(out=b2cat[:, pi:pi + 1], in_=w[f"{pre}_b2"][:, None])
            slot3 = 0
            for pre, nout in _MLP_PRES:
                for i in range(nout):
                    wdma(out=b3cat[:, slot3:slot3 + 1],
                         in_=w[f"{pre}_b3"][i * D:(i + 1) * D][:, None])
                    slot3 += 1
                mlp_w[pre] = tiles[pre]
            mlp_w["biases"] = (b1cat, b2cat, b3cat)

        mlp_out = {}

        def emit_mlps():
            """All 4 cond MLPs batched: layer k runs as 4 (or 6) matmuls
            into one [128, 32/48] PSUM tile + ONE batched silu chain, so
            the serial cross-engine chain is 3 layers deep instead of 12.
            silu(z) = z / (1 + exp(-z)); only Exp touches ACT."""
            b1cat, b2cat, b3cat = mlp_w["biases"]

            def layer(rhs_of, nin_cols, bias_cat, n_pre_cols, w_idx):
                mm = mlp_ps.tile([D, 8 * n_pre_cols], F32, tag="mlp",
                                 name=f"mlp_mm{w_idx}")
                col = 0
                for pi, (pre, nout) in enumerate(_MLP_PRES):
                    ws = mlp_w[pre][w_idx]
                    nslice = nout if w_idx == 2 else 1
                    for i in range(nslice):
                        nc.tensor.matmul(
                            out=mm[:, col * SPC:(col + 1) * SPC],
                            lhsT=ws[:, i * D:(i + 1) * D],
                            rhs=rhs_of(pi),
                            skip_group_check=True)
                        col += 1
                z = mlp_tmp.tile([D, 8 * n_pre_cols], F32, tag=f"z{w_idx}",
                                 name=f"mlp_z{w_idx}")
                nc.vector.scalar_tensor_tensor(
                    out=z.rearrange("p (c s) -> p c s", s=SPC),
                    in0=mm.rearrange("p (c s) -> p c s", s=SPC),
                    scalar=1.0,
                    in1=bias_cat[:, :, None].broadcast_to((D, n_pre_cols, SPC)),
                    op0=ALU.mult, op1=ALU.add)
                return z

            def silu(z, tag):
                nf = z.shape[1]
                e = mlp_tmp.tile([D, nf], F32, tag=f"e{tag}", name=f"mlp_e{tag}")
                nc.scalar.activation(out=e, in_=z, func=AF.Exp, scale=-1.0)
                sp1 = mlp_tmp.tile([D, nf], F32, tag=f"sp{tag}",
                                   name=f"mlp_sp{tag}")
                nc.vector.tensor_scalar_add(out=sp1, in0=e, scalar1=1.0)
                r = mlp_tmp.tile([D, nf], F32, tag=f"r{tag}", name=f"mlp_r{tag}")
                nc.vector.reciprocal_approx_fast(out=r, in_=sp1)
                h = mlp_tmp.tile([D, nf], F32, tag=f"h{tag}", name=f"mlp_h{tag}")
                nc.vector.tensor_mul(out=h, in0=z, in1=r)
                return h

            z1 = layer(lambda pi: condT, 4, b1cat, 4, 0)
            warm_pe(3)
            h1 = silu(z1, 1)
            z2 = layer(lambda pi: h1[:, pi * SPC:(pi + 1) * SPC], 4, b2cat, 4, 1)
            warm_pe(3)
            h2 = silu(z2, 2)
            adaln = layer(lambda pi: h2[:, pi * SPC:(pi + 1) * SPC], 4, b3cat, 6, 2)
            # faithful reference bug: (alpha, gamma, beta) <- (g, be, al)
            mlp_out["a1"] = adaln[:, 0:8]
            mlp_out["g1"] = adaln[:, 8:16]
            mlp_out["b1"] = adaln[:, 16:24]
            mlp_out["a2"] = adaln[:, 24:32]
            mlp_out["g2"] = adaln[:, 32:40]
            mlp_out["b2"] = adaln[:, 40:48]

        # ================= per-sample state =================
        xts = [None] * SPC
        xcs = [None] * SPC
        x2s = [None] * SPC
        qts = [None] * SPC
        kts = [None] * SPC
        vs = [None] * SPC
        x1s = [None] * SPC
        xc2s = [None] * SPC
        lnp1 = [None] * (SPC // 2)
        lnp2 = [None] * (SPC // 2)
        rstd1 = [None] * SPC
        rstd2 = [None] * SPC

        pre_sums = {}

        def emit_sum_mm(x_r, key):
            """Early-emitted f32r sum matmul (PE-only, no DVE dependency)
            so it can run during the DMA-bound startup window."""
            sum_ps = sp.tile([D, N], F32, tag="sp", name=f"presum_{key}")
            nc.tensor.matmul(out=sum_ps, lhsT=onesmat_r, rhs=x_r)
            pre_sums[key] = sum_ps

        def snorm_stats(x_r, lnp_tile, half, f32r_sum=False, pre_key=None):
            """sum/var stats for one sample; writes ln(v) into lnp half.
            The sum matmul normally contracts a bf16 copy of x (f32r
            streams at half rate on the PE); in latency-critical phases
            (startup / drain) f32r_sum=True skips the copy hop instead.
            """
            if pre_key is not None:
                sum_ps = pre_sums.pop(pre_key)
            elif f32r_sum:
                sum_ps = sp.tile([D, N], F32, tag="sp")
                nc.tensor.matmul(out=sum_ps, lhsT=onesmat_r, rhs=x_r)
            else:
                xb = xsq_p.tile([D, N], BF16, tag="xb")
                nc.vector.tensor_copy(out=xb, in_=x_r.bitcast(F32))
                sum_ps = sp.tile([D, N], F32, tag="sp")
                nc.tensor.matmul(out=sum_ps, lhsT=ones_bf, rhs=xb)
            xc = xc_p.tile([D, N], BF16, tag="xc")
            nc.vector.scalar_tensor_tensor(
                out=xc, in0=sum_ps, scalar=-1.0 / D, in1=x_r.bitcast(F32),
                op0=ALU.mult, op1=ALU.add)
            xcsq = xsq_p.tile([D, N], BF16, tag="xcsq")
            nc.vector.tensor_mul(out=xcsq, in0=xc, in1=xc)
            s2_ps = sp.tile([D, N], F32, tag="sp")
            nc.tensor.matmul(out=s2_ps, lhsT=ones_bf, rhs=xcsq)
            nc.scalar.activation(out=lnp_tile[:, half * N:(half + 1) * N],
                                 in_=s2_ps, func=AF.Ln, scale=1.0 / (D - 1))
            return xc

        def rstd_pair(lnp_tile, tag):
            """rstd = exp(-0.5 ln v) for a sample pair in one ACTIVATE."""
            r = rstd_p.tile([D, 2 * N], BF16, tag=tag)
            nc.scalar.activation(out=r, in_=lnp_tile, func=AF.Exp, scale=-0.5)
            return r

        def rstd_single(lnp_tile, half, tag):
            """per-sample rstd (shorter dependency chain at pipeline edges)"""
            r = rstd_p.tile([D, N], BF16, tag=tag)
            nc.scalar.activation(out=r, in_=lnp_tile[:, half * N:(half + 1) * N],
                                 func=AF.Exp, scale=-0.5)
            return r

        def prep_a_closures(j, f32r_sum=False, paired=True):
            """snorm1 stats + rstd for sample pair j, as pump-able steps."""
            s0 = 2 * j

            def c_stats(s=s0):
                if s == s0:
                    lnp1[j] = lnp_p.tile([D, 2 * N], F32, tag="lnp1",
                                         name=f"lnp1_{j}")
                pk = f"p1_{s}" if f"p1_{s}" in pre_sums else None
                xcs[s] = snorm_stats(xts[s], lnp1[j], s % 2,
                                     f32r_sum=f32r_sum, pre_key=pk)

            def c_rstd():
                r = rstd_pair(lnp1[j], "rstd1")
                rstd1[s0] = r[:, 0:N]
                rstd1[s0 + 1] = r[:, N:2 * N]

            def c_rstd_one(s):
                rstd1[s] = rstd_single(lnp1[j], s % 2, "rstd1s")

            if paired:
                return [lambda s=s0: c_stats(s), lambda s=s0 + 1: c_stats(s),
                        c_rstd]
            return [lambda s=s0: c_stats(s), lambda s=s0: c_rstd_one(s),
                    lambda s=s0 + 1: c_stats(s),
                    lambda s=s0 + 1: c_rstd_one(s)]

        def prep_b_closures(j):
            """x2 build + qkv staging for sample pair j, pump-able."""
            s0 = 2 * j

            def c_x2(s):
                rs = rstd1[s]
                xhat = xh_p.tile([D, N], BF16, tag="xh", name=f"xh_{s}")
                nc.vector.tensor_mul(out=xhat, in0=xcs[s], in1=rs)
                x2 = x2_p.tile([D, N], BF16, tag="x2", name=f"x2_{s}")
                nc.vector.tensor_scalar(
                    out=x2, in0=xhat,
                    scalar1=mlp_out["g1"][:, s:s + 1],
                    scalar2=mlp_out["b1"][:, s:s + 1],
                    op0=ALU.mult, op1=ALU.add)
                x2s[s] = x2

            def c_qt(s):
                qt_ps = sp.tile([D, N], F32, tag="sp", name=f"qt_ps_{s}")
                nc.tensor.matmul(out=qt_ps, lhsT=qkv_w["qw"], rhs=x2s[s])
                qt = qt_p.tile([D, N], BF16, tag="qt", name=f"qt_{s}")
                nc.vector.tensor_scalar_add(out=qt, in0=qt_ps,
                                            scalar1=qkv_w["qb"])
                qts[s] = qt

            def c_kt(s):
                kt_ps = sp.tile([D, N], F32, tag="sp", name=f"kt_ps_{s}")
                nc.tensor.matmul(out=kt_ps, lhsT=qkv_w["kw"], rhs=x2s[s])
                kt = kt_p.tile([D, N], BF16, tag="kt", name=f"kt_{s}")
                nc.vector.tensor_scalar_add(out=kt, in0=kt_ps,
                                            scalar1=qkv_w["kb"])
                kts[s] = kt

            def c_v(s):
                x2 = x2s[s]
                vp_ps = sp.tile([D, N], F32, tag="sp", name=f"vp_ps_{s}")
                for c in range(4):
                    nc.tensor.matmul(out=vp_ps[:, c * NC:(c + 1) * NC],
                                     lhsT=x2[:, c * NC:(c + 1) * NC],
                                     rhs=qkv_w["vw"])
                v_sb = v_p.tile([D, N], BF16, tag="v", name=f"v_{s}")
                nc.vector.scalar_tensor_tensor(
                    out=v_sb.rearrange("p (c k) -> p c k", c=4),
                    in0=vp_ps.rearrange("p (c k) -> p c k", c=4),
                    scalar=1.0,
                    in1=qkv_w["vb_b"][:, None, :].broadcast_to((D, 4, D)),
                    op0=ALU.mult, op1=ALU.add)
                vs[s] = v_sb

            out = []
            for s in (s0, s0 + 1):
                out += [lambda s=s: c_x2(s), lambda s=s: c_qt(s),
                        lambda s=s: c_kt(s), lambda s=s: c_v(s)]
            return out

        # (c_v kept fused: the 4 vp matmuls + stt form one dependency unit)

        def fin_closures(j, f32r_sum=False):
            """snorm2 + final residual + store for pair j, pump-able."""
            s0 = 2 * j

            def c_stats2(s):
                if s == s0:
                    lnp2[j] = lnp_p.tile([D, 2 * N], F32, tag="lnp2",
                                         name=f"lnp2_{j}")
                xc2s[s] = snorm_stats(x1s[s], lnp2[j], s % 2,
                                      f32r_sum=f32r_sum)

            def c_rstd2():
                r = rstd_pair(lnp2[j], "rstd2")
                rstd2[s0] = r[:, 0:N]
                rstd2[s0 + 1] = r[:, N:2 * N]

            def c_xf(s):
                rs = rstd2[s]
                xhat2 = xh_p.tile([D, N], BF16, tag="xh", name=f"xh2_{s}")
                nc.vector.tensor_mul(out=xhat2, in0=xc2s[s], in1=rs)
                x2b = x2_p.tile([D, N], BF16, tag="x2", name=f"x2b_{s}")
                nc.vector.tensor_scalar(
                    out=x2b, in0=xhat2,
                    scalar1=mlp_out["g2"][:, s:s + 1],
                    scalar2=mlp_out["b2"][:, s:s + 1],
                    op0=ALU.mult, op1=ALU.add)
                xf = xf_p.tile([D, N], F32, tag="xf", name=f"xf_{s}")
                nc.vector.scalar_tensor_tensor(
                    out=xf, in0=x2b, scalar=mlp_out["a2"][:, s:s + 1],
                    in1=x1s[s].bitcast(F32), op0=ALU.mult, op1=ALU.add)
                dma(out=out2[s], in_=xf)

            return [lambda s=s0: c_stats2(s), lambda s=s0 + 1: c_stats2(s),
                    c_rstd2, lambda s=s0: c_xf(s), lambda s=s0 + 1: c_xf(s)]

        # Background-work pump: prep/fin steps are emitted interleaved into
        # the attention chunk stream so their PE/DVE/ACT work executes in
        # the slack while ACT grinds through the exps (instead of as a
        # serial burst between attention phases that stalls the exp queue).
        from collections import deque
        bg = deque()

        def pump(k=1, warm_fallback=False):
            for _ in range(k):
                if bg:
                    bg.popleft()()
                elif warm_fallback:
                    warm_pe(1)
                    warm_fallback = False

        first_st = {}

        def attn(s):
            """attention + out-proj + residual for one sample.

            Half-chunk (2-head) pipeline: while ACT runs exp on one
            [128,1024] S^T half-tile, the PE retires the previous half's
            P@V + denominator MMs and computes the next half's S^T into
            the other buffer, so ACT stays near-saturated.
            """
            qt, kt, v_sb = qts[s], kts[s], vs[s]
            pv = pv_p.tile([D, N], F32, tag="pv", name=f"pv_{s}")
            den = den_p.tile([D, N], F32, tag="den", name=f"den_{s}")

            def st_chunk(c):
                st4 = st4_holder["p"].tile([D, H * N], F32, tag="st4",
                                           name=f"st4_{s}_{c}")
                for h in range(H):
                    nc.tensor.matmul(
                        out=st4[:, h * N:(h + 1) * N],
                        lhsT=kt[h * DK:(h + 1) * DK, c * NC:(c + 1) * NC],
                        rhs=qt[h * DK:(h + 1) * DK, :],
                        tile_position=(h * DK, 0))
                return st4

            def pv_den_chunk(c, est):
                for h in range(H):
                    nc.tensor.matmul(
                        out=pv[h * DK:(h + 1) * DK, :],
                        lhsT=v_sb[:, c * NC + h * DK:c * NC + (h + 1) * DK],
                        rhs=est[:, h * N:(h + 1) * N],
                        start=(c == 0), stop=(c == 3),
                        tile_position=(0, h * DK),
                        skip_group_check=True)
                for h in range(H):
                    nc.tensor.matmul(
                        out=den[h * DK:(h + 1) * DK, :],
                        lhsT=ones_bf[:, 0:DK],
                        rhs=est[:, h * N:(h + 1) * N],
                        start=(c == 0), stop=(c == 3),
                        tile_position=(0, h * DK),
                        skip_group_check=True)

            st4 = first_st.pop(s, None)
            if st4 is None:
                st4 = st_chunk(0)
            for c in range(4):
                est = est_p.tile([D, H * N], BF16, tag="est",
                                 name=f"est_{s}_{c}")
                nc.scalar.activation(out=est, in_=st4, func=AF.Exp,
                                     scale=Q_SCALE)
                # next chunk's S^T goes into the PE queue BEFORE this
                # chunk's P@V/den so exp(c+1) can start right after exp(c);
                # P@V/den then execute under exp(c+1)'s shadow. On the last
                # chunk, hoist the NEXT sample's first S^T instead so the
                # sample boundary costs the same single-S^T bubble.
                if c < 3:
                    st4 = st_chunk(c + 1)
                elif s + 1 < SPC and qts[s + 1] is not None:
                    qt, kt = qts[s + 1], kts[s + 1]
                    first_st[s + 1] = st_chunk(0)
                pump(1)
                pv_den_chunk(c, est)
                pump(1)

            def tail():
                rd = rd_p.tile([D, N], F32, tag="rd", name=f"rd_{s}")
                nc.vector.reciprocal_approx_fast(out=rd, in_=den)
                o_all = oall_p.tile([D, N], BF16, tag="oall",
                                    name=f"oall_{s}")
                nc.vector.tensor_mul(out=o_all, in0=pv, in1=rd)
                attn_ps = sp.tile([D, N], F32, tag="sp", name=f"attn_ps_{s}")
                nc.tensor.matmul(out=attn_ps, lhsT=qkv_w["ow"], rhs=o_all)
                x1 = x1_p.tile([D, N], F32R, tag="x1", name=f"x1_{s}")
                nc.vector.scalar_tensor_tensor(
                    out=x1, in0=attn_ps, scalar=mlp_out["a1"][:, s:s + 1],
                    in1=xts[s].bitcast(F32), op0=ALU.mult, op1=ALU.add)
                x1s[s] = x1

            # normalize/project/residual runs as background work inside the
            # NEXT sample's attention so its DVE chain + ow matmul never
            # block the next S^T -> exp stream.
            bg.appendleft(tail)

        # ============== emission schedule ==============
        def run_all(cs):
            for c in cs:
                c()

        load_mlp_weights()
        emit_sum_mm(xts[0], "p1_0")
        emit_sum_mm(xts[1], "p1_1")
        emit_mlps()
        run_all(prep_a_closures(0, f32r_sum=True, paired=False))
        mlp_ps_cm.__exit__(None, None, None)
        st4_holder["p"] = ctx.enter_context(
            tc.tile_pool(name="st4", bufs=1, space="PSUM"))
        load_qkv_weights()
        run_all(prep_b_closures(0))
        for s in range(4, SPC):
            load_xt(s)
        bg.extend(prep_a_closures(1, f32r_sum=True) + prep_b_closures(1)
                  + prep_a_closures(2) + prep_b_closures(2))
        attn(0)
        attn(1)
        bg.extend(fin_closures(0) + prep_a_closures(3) + prep_b_closures(3))
        attn(2)
        attn(3)
        bg.extend(fin_closures(1))
        attn(4)
        fin2 = fin_closures(2)
        bg.extend(fin2[:1])
        attn(5)
        bg.extend(fin2[1:])
        attn(6)
        fin3 = fin_closures(3, f32r_sum=True)
        bg.extend(fin3[:1])
        attn(7)
        while bg:
            pump(1)
        run_all(fin3[1:])


_NC_CACHE = None


def _get_program():
    global _NC_CACHE
    if _NC_CACHE is None:
        _NC_CACHE = build_program()
    return _NC_CACHE


def _shard_inputs(inputs):
    in_maps = []
    for c in range(NCORES):
        m = {}
        lo = c * SPC
        m["latent"] = np.ascontiguousarray(inputs["latent"][lo:lo + SPC], dtype=np.float32)
        m["nodes"] = np.ascontiguousarray(inputs["nodes"][lo:lo + SPC], dtype=np.float32)
        m["t"] = np.ascontiguousarray(inputs["t"][lo:lo + SPC], dtype=np.float32)
        for nm in _WEIGHT_NAMES:
            m[nm] = np.ascontiguousarray(inputs[nm], dtype=np.float32)
        in_maps.append(m)
    return in_maps


def _run(inputs, trace=False, tmpdir=None):
    nc = _get_program()
    in_maps = _shard_inputs(inputs)
    res = run_bass_kernel_spmd(nc, in_maps, list(range(NCORES)), trace=trace,
                               tmpdir=tmpdir)
    outs = [res.results[c]["out"] for c in range(NCORES)]
    full = np.concatenate(outs, axis=0).astype(np.float32)
    return full, res.exec_time_ns


def kernel(**inputs):
    full, _ = _run(inputs, trace=False)
    return full

